# revision 7
# baseline (speedup 1.0000x reference)
"""Trainium2 Bass kernel for nn_LstmEncDeltaAllHistStacked (v3).

v3 = v2 (1-core, For_i edge loop, persistent compile cache) plus:
  * all inputs packed into ONE DRAM tensor (single device_put per call)
  * node/seq/dec LSTMs also run as hardware loops (8 iters each), with
    h-chain buffers so the s==0 special case disappears (h_prev = 0)
  * pose head as 8 accumulating K=32 matmuls over the dec h-chain
    (replaces the partition-stacked DECP0/DECP1 layout)

Packed input layout (fp32, one row-major [1, NTOT] buffer; offsets in
elements, hardcoded to match _pack_inputs):
  scene_js [2, 2048], scene_sp [3, 2048], w_node_x [3, 256],
  w_node_h [64, 256], w_edge [67, 256], w_seq_x [65, 256],
  w_seq_h [64, 256], w_dec_x [128, 128], w_dec_h [33, 128],
  w_pose_s [32, 16] (col s*2+d = pose_W.T[s-block]), pose_b2 [2, 1]
"""

import os
import numpy as np

NP, SEQ, D, H, EMB = 256, 8, 2, 64, 32
NCORES = 1
PPC = NP
B = PPC * SEQ           # 2048
G4 = 4 * H              # 256
GD = 4 * EMB            # 128
CHUNK = 512
NCH = B // CHUNK        # 4

# packed layout: (name, rows, cols)
_PACK = [
    ("scene_js", D, B),
    ("scene_sp", D + 1, B),
    ("w_node_x", 3, G4),
    ("w_node_h", H, G4),
    ("w_edge", H + 3, G4),
    ("w_seq_x", H + 1, G4),
    ("w_seq_h", H, G4),
    ("w_dec_x", 2 * H, GD),
    ("w_dec_h", EMB + 1, GD),
    ("w_pose_s", EMB, 2 * SEQ),
    ("pose_b2", D, 1),
]
_OFFS = {}
_off = 0
for _n, _r, _c in _PACK:
    _OFFS[_n] = _off
    _off += _r * _c
NTOT = _off

_CACHE = {}


def _enable_jax_compile_cache():
    """Persistent XLA compile cache: run_bass_kernel_spmd rebuilds its jit
    closure per call, so without this every call re-runs the full BIR->NEFF
    compile (~250ms).  Standard jax feature; safe no-op if unavailable."""
    try:
        import jax

        cache_dir = "/tmp/jax_cc_cache"
        os.makedirs(cache_dir, exist_ok=True)
        jax.config.update("jax_compilation_cache_dir", cache_dir)
        jax.config.update("jax_persistent_cache_min_entry_size_bytes", -1)
        jax.config.update("jax_persistent_cache_min_compile_time_secs", 0.0)
    except Exception:
        pass


_enable_jax_compile_cache()


def _build_nc():
    import concourse.bass as bass
    import concourse.tile as tile
    from concourse import bacc, mybir

    f32 = mybir.dt.float32
    AF = mybir.ActivationFunctionType
    OP = mybir.AluOpType

    nc = bacc.Bacc("TRN2", target_bir_lowering=False, debug=False)

    packed_d = nc.dram_tensor("packed_in", [1, NTOT], f32, kind="ExternalInput")
    out_d = nc.dram_tensor("tag_t", [D, PPC], f32, kind="ExternalOutput")

    def pk(name, rows, cols):
        o = _OFFS[name]
        return packed_d[0, o : o + rows * cols].rearrange("(r c) -> r c", c=cols)

    with tile.TileContext(nc) as tc:
        with (
            tc.tile_pool(name="const", bufs=1) as cpool,
            tc.tile_pool(name="state", bufs=1) as spool,
            tc.tile_pool(name="tmp_e", bufs=1) as epool,
            tc.tile_pool(name="tmp_s", bufs=2) as tpool,
        ):
            # ---- load constants (from the packed buffer) ----
            WNX = cpool.tile([3, G4], f32)
            WNH = cpool.tile([H, G4], f32)
            WE = cpool.tile([H + 3, G4], f32)
            WSX = cpool.tile([H + 1, G4], f32)
            WSH = cpool.tile([H, G4], f32)
            WDX = cpool.tile([2 * H, GD], f32)
            WDH = cpool.tile([EMB + 1, GD], f32)
            WPS = cpool.tile([EMB, 2 * SEQ], f32)
            PB = cpool.tile([D, 1], f32)
            SJS = cpool.tile([D, B], f32)
            SLOCE = cpool.tile([3, B], f32)
            for t, (name, rows, cols) in zip(
                [SJS, SLOCE, WNX, WNH, WE, WSX, WSH, WDX, WDH, WPS, PB], _PACK
            ):
                nc.sync.dma_start(t[:], pk(name, rows, cols))

            # ---- persistent state ----
            # CAT rows 0:64 node h (lstm_out), rows 64:128 seq h (full_dist)
            CAT = spool.tile([2 * H, B], f32)
            RHSE = spool.tile([H + 3, B], f32)   # edge rhs: h | x | ones
            EDGEHE = spool.tile([H + 1, B], f32)  # dist_hist | ones
            NODEH = spool.tile([H, (SEQ + 1) * PPC], f32)  # node h chain
            SEQH = spool.tile([H, (SEQ + 1) * PPC], f32)   # seq h chain
            CN = spool.tile([2 * H, PPC], f32)  # c in rows 64:128
            CE = spool.tile([2 * H, B], f32)
            CS = spool.tile([2 * H, PPC], f32)
            CD = spool.tile([4 * EMB, PPC], f32)  # c in rows 32:64
            RHSD = spool.tile([EMB + 1, (SEQ + 1) * PPC], f32)  # dec h | ones
            NEGSLOC = cpool.tile([D, B], f32)

            nc.scalar.mul(NEGSLOC[:], SLOCE[0:2, :], -1.0)
            nc.gpsimd.memset(RHSE[0:H, :], 0.0)
            nc.sync.dma_start(RHSE[H + 2 : H + 3, :], SLOCE[2:3, :])
            nc.gpsimd.memset(EDGEHE[H : H + 1, :], 1.0)
            nc.gpsimd.memset(NODEH[:, 0:PPC], 0.0)
            nc.gpsimd.memset(SEQH[:, 0:PPC], 0.0)
            nc.gpsimd.memset(CN[H : 2 * H, :], 0.0)
            nc.gpsimd.memset(CE[H : 2 * H, :], 0.0)
            nc.gpsimd.memset(CS[H : 2 * H, :], 0.0)
            nc.gpsimd.memset(CD[EMB : 2 * EMB, :], 0.0)
            nc.gpsimd.memset(RHSD[:, 0:PPC], 0.0)
            nc.gpsimd.memset(RHSD[EMB : EMB + 1, :], 1.0)

            def small_lstm_loop(WX, WH, HCH, Cst, xs_of, tag):
                """8-step LSTM as a hardware loop; h chain in HCH
                ([H, 9*PPC], slice 0 zeroed), gates via the all-sigmoid
                trick.  xs_of(iv) -> x-slice [Kx, PPC] for step iv//PPC."""
                S = tpool.tile([2 * H, 2 * PPC], f32, tag=tag + "s")
                Q = tpool.tile([2 * H, PPC], f32, tag=tag + "q")
                P1 = tpool.tile([2 * H, PPC], f32, tag=tag + "p1")
                P2 = tpool.tile([2 * H, PPC], f32, tag=tag + "p2")
                TH = tpool.tile([2 * H, PPC], f32, tag=tag + "th")
                GP = tpool.tile_psum([2 * H, 2 * PPC], f32, tag=tag + "g")
                c = Cst[H : 2 * H, :]
                with tc.For_i(0, SEQ * PPC, PPC) as iv:
                    rx = xs_of(iv)
                    rh = HCH[:, bass.ds(iv, PPC)]
                    for mh in range(2):
                        o = GP[:, mh * PPC : (mh + 1) * PPC]
                        nc.tensor.matmul(
                            o, WX[:, mh * 128 : (mh + 1) * 128], rx,
                            start=True, stop=False,
                        )
                        nc.tensor.matmul(
                            o, WH[:, mh * 128 : (mh + 1) * 128], rh,
                            start=False, stop=True,
                        )
                    nc.scalar.activation(S[:], GP[:], AF.Sigmoid)
                    si, sf = S[0:H, 0:PPC], S[H : 2 * H, 0:PPC]
                    sg = S[0:H, PPC : 2 * PPC]
                    so = S[H : 2 * H, PPC : 2 * PPC]
                    nc.vector.tensor_mul(Q[0:H, :], si, sg)
                    nc.vector.scalar_tensor_tensor(
                        P1[0:H, :], Q[0:H, :], 2.0, si,
                        op0=OP.mult, op1=OP.subtract,
                    )
                    nc.vector.tensor_mul(P2[0:H, :], sf, c)
                    nc.vector.tensor_add(c, P1[0:H, :], P2[0:H, :])
                    nc.scalar.activation(TH[H : 2 * H, :], c, AF.Tanh)
                    nc.vector.tensor_mul(
                        HCH[:, bass.ds(iv + PPC, PPC)], so, TH[H : 2 * H, :]
                    )

            # ======== node LSTM (batch 256, hw loop over 8 steps) ========
            with tc.tile_pool(
                name="ps_n", bufs=1, space=bass.MemorySpace.PSUM
            ) as ps_n:
                tpool.tile_psum = (
                    lambda shape, dt, tag: ps_n.tile(shape, dt, tag=tag, name=tag)
                )
                small_lstm_loop(
                    WNX, WNH, NODEH, CN,
                    lambda iv: SLOCE[:, bass.ds(iv, PPC)], "n",
                )
                # lstm_out -> CAT rows 0:64
                nc.vector.tensor_copy(CAT[0:H, :], NODEH[:, PPC:])

            # ======== edge LSTM (batch 2048, hw loop over 256 steps) =====
            with tc.tile_pool(
                name="ps_e", bufs=1, space=bass.MemorySpace.PSUM
            ) as ps_e:
                GE = ps_e.tile([2 * H, 2 * B], f32)
                S = epool.tile([2 * H, 2 * B], f32)
                Q = epool.tile([2 * H, B], f32)
                P1 = epool.tile([2 * H, B], f32)
                P2 = epool.tile([2 * H, B], f32)
                TH = epool.tile([2 * H, B], f32)
                c = CE[H : 2 * H, :]
                with tc.For_i(0, B, SEQ) as iv:
                    nc.vector.tensor_add(
                        RHSE[H : H + 2, :].rearrange("d (s p) -> d s p", p=PPC),
                        SJS[:, bass.ds(iv, SEQ)]
                        .unsqueeze(2)
                        .broadcast_to((D, SEQ, PPC)),
                        NEGSLOC[:].rearrange("d (s p) -> d s p", p=PPC),
                    )
                    for ch in range(NCH):
                        rc = RHSE[:, ch * CHUNK : (ch + 1) * CHUNK]
                        for mh in range(2):
                            nc.tensor.matmul(
                                GE[:, mh * B + ch * CHUNK : mh * B + (ch + 1) * CHUNK],
                                WE[:, mh * 128 : (mh + 1) * 128],
                                rc,
                                start=True, stop=True,
                            )
                    nc.scalar.activation(S[:], GE[:], AF.Sigmoid)
                    si, sf = S[0:H, 0:B], S[H : 2 * H, 0:B]
                    sg, so = S[0:H, B : 2 * B], S[H : 2 * H, B : 2 * B]
                    nc.vector.tensor_mul(Q[0:H, :], si, sg)
                    nc.vector.scalar_tensor_tensor(
                        P1[0:H, :], Q[0:H, :], 2.0, si,
                        op0=OP.mult, op1=OP.subtract,
                    )
                    nc.vector.tensor_mul(P2[0:H, :], sf, c)
                    nc.vector.tensor_add(c, P1[0:H, :], P2[0:H, :])
                    nc.scalar.activation(TH[H : 2 * H, :], c, AF.Tanh)
                    nc.vector.tensor_mul(RHSE[0:H, :], so, TH[H : 2 * H, :])
                nc.vector.tensor_copy(EDGEHE[0:H, :], RHSE[0:H, :])

            # ======== seq LSTM (batch 256, hw loop over 8 steps) =========
            with tc.tile_pool(
                name="ps_s", bufs=1, space=bass.MemorySpace.PSUM
            ) as ps_s:
                tpool.tile_psum = (
                    lambda shape, dt, tag: ps_s.tile(shape, dt, tag=tag, name=tag)
                )
                small_lstm_loop(
                    WSX, WSH, SEQH, CS,
                    lambda iv: EDGEHE[:, bass.ds(iv, PPC)], "e",
                )
                # full_dist -> CAT rows 64:128 (partition remap via DMA)
                nc.sync.dma_start(CAT[H : 2 * H, :], SEQH[:, PPC:])

                # ======== decoder LSTM (hw loop, H=EMB=32) ========
                SD = tpool.tile([4 * EMB, PPC], f32, tag="dsif")
                TGSO = tpool.tile([4 * EMB, PPC], f32, tag="dtgso")
                DP1 = tpool.tile([4 * EMB, PPC], f32, tag="dp1")
                DP2 = tpool.tile([4 * EMB, PPC], f32, tag="dp2")
                DTH = tpool.tile([4 * EMB, PPC], f32, tag="dth")
                GDm = ps_s.tile([GD, PPC], f32, tag="gdec")
                cd = CD[EMB : 2 * EMB, :]
                with tc.For_i(0, SEQ * PPC, PPC) as iv:
                    nc.tensor.matmul(
                        GDm[:], WDX[:], CAT[:, bass.ds(iv, PPC)],
                        start=True, stop=False,
                    )
                    nc.tensor.matmul(
                        GDm[:], WDH[:], RHSD[:, bass.ds(iv, PPC)],
                        start=False, stop=True,
                    )
                    nc.scalar.activation(
                        SD[0 : 2 * EMB, :], GDm[0 : 2 * EMB, :], AF.Sigmoid
                    )
                    nc.scalar.activation(
                        TGSO[0:EMB, :], GDm[2 * EMB : 3 * EMB, :], AF.Tanh
                    )
                    nc.scalar.activation(
                        TGSO[EMB : 2 * EMB, :], GDm[3 * EMB : 4 * EMB, :],
                        AF.Sigmoid,
                    )
                    nc.vector.tensor_mul(
                        DP1[0:EMB, :], SD[0:EMB, :], TGSO[0:EMB, :]
                    )
                    nc.vector.tensor_mul(
                        DP2[0:EMB, :], SD[EMB : 2 * EMB, :], cd
                    )
                    nc.vector.tensor_add(cd, DP1[0:EMB, :], DP2[0:EMB, :])
                    nc.scalar.activation(DTH[EMB : 2 * EMB, :], cd, AF.Tanh)
                    nc.vector.tensor_mul(
                        RHSD[0:EMB, bass.ds(iv + PPC, PPC)],
                        TGSO[EMB : 2 * EMB, :],
                        DTH[EMB : 2 * EMB, :],
                    )

                # ======== pose head: 8 accumulating K=32 matmuls ========
                TAGT = ps_s.tile([D, PPC], f32, tag="tag")
                for s in range(SEQ):
                    nc.tensor.matmul(
                        TAGT[:],
                        WPS[:, 2 * s : 2 * (s + 1)],
                        RHSD[0:EMB, (s + 1) * PPC : (s + 2) * PPC],
                        start=(s == 0), stop=(s == SEQ - 1),
                    )
                OUTT = tpool.tile([D, PPC], f32, tag="outt")
                nc.vector.scalar_tensor_tensor(
                    OUTT[:], TAGT[:], PB[:],
                    SLOCE[0:2, (SEQ - 1) * PPC : SEQ * PPC],
                    op0=OP.add, op1=OP.add,
                )
                nc.sync.dma_start(out_d[:], OUTT[:])

    nc.compile()
    return nc


def _prep_weights(i):
    """Host-side constant folding of the LSTM weights into matmul layouts."""
    c = np.concatenate
    f = np.float32
    wnx = c([i["node_Wih"].T, (i["node_bih"] + i["node_bhh"])[None]], 0).copy()
    wnh = i["node_Whh"].T.copy()
    wnx[:, 128:192] *= 2.0
    wnh[:, 128:192] *= 2.0
    we = c([i["edge_Whh"].T, i["edge_Wih"].T,
            (i["edge_bih"] + i["edge_bhh"])[None]], 0)
    we = we.copy()
    we[:, 128:192] *= 2.0  # g-gate cols: tanh(g) = 2*sigmoid(2g) - 1
    wsx = c([i["seq_Wih"].T, (i["seq_bih"] + i["seq_bhh"])[None]], 0).copy()
    wsh = i["seq_Whh"].T.copy()
    wsx[:, 128:192] *= 2.0
    wsh[:, 128:192] *= 2.0
    wdx = i["dec_Wih"].T
    wdh = c([i["dec_Whh"].T, (i["dec_bih"] + i["dec_bhh"])[None]], 0)
    # pose_W [2, 256] -> per-step blocks: w_pose_s[e, s*2+d] = pose_W[d, s*32+e]
    wps = np.ascontiguousarray(
        i["pose_W"].reshape(2, SEQ, EMB).transpose(2, 1, 0).reshape(EMB, 2 * SEQ)
    )
    pb = i["pose_b"][:, None]
    return {
        "w_node_x": np.ascontiguousarray(wnx, f),
        "w_node_h": np.ascontiguousarray(wnh, f),
        "w_edge": np.ascontiguousarray(we, f),
        "w_seq_x": np.ascontiguousarray(wsx, f),
        "w_seq_h": np.ascontiguousarray(wsh, f),
        "w_dec_x": np.ascontiguousarray(wdx, f),
        "w_dec_h": np.ascontiguousarray(wdh, f),
        "w_pose_s": wps.astype(f),
        "pose_b2": np.ascontiguousarray(pb, f),
    }


def make_in_maps(**inputs):
    scene = np.ascontiguousarray(np.asarray(inputs["scene"], np.float32))
    w = _prep_weights({k: np.asarray(v, np.float32) for k, v in inputs.items()})
    w["scene_js"] = np.ascontiguousarray(scene.transpose(2, 0, 1).reshape(D, B))
    ssp = scene.transpose(2, 1, 0).reshape(D, B)
    w["scene_sp"] = np.ascontiguousarray(
        np.concatenate([ssp, np.ones((1, B), np.float32)], 0)
    )
    packed = np.empty((1, NTOT), np.float32)
    for name, rows, cols in _PACK:
        o = _OFFS[name]
        packed[0, o : o + rows * cols] = w[name].reshape(-1)
    return [{"packed_in": packed}]


def gather_out(results):
    out = np.zeros((NP, 1, D), np.float32)
    out[:, 0, :] = results[0]["tag_t"].T
    return out


def kernel(**inputs):
    from concourse.bass_utils import run_bass_kernel_spmd

    first = "nc" not in _CACHE
    if first:
        _CACHE["nc"] = _build_nc()
    nc = _CACHE["nc"]
    in_maps = make_in_maps(**inputs)
    res = run_bass_kernel_spmd(nc, in_maps, list(range(NCORES)))
    if first:
        # Warm the dispatch path (compile cache, executable-load dedup in
        # the PJRT client/terminal): per-call latency settles only after a
        # few calls in a fresh process.
        for _ in range(4):
            run_bass_kernel_spmd(nc, in_maps, list(range(NCORES)))
    return gather_out(res.results)


if __name__ == "__main__":
    rng = np.random.default_rng(0)
    dummy = {}
    dummy["scene"] = rng.normal(size=(NP, SEQ, D)).astype(np.float32)
    for n, s in [
        ("node_Wih", (G4, D)), ("node_Whh", (G4, H)),
        ("node_bih", (G4,)), ("node_bhh", (G4,)),
        ("edge_Wih", (G4, D)), ("edge_Whh", (G4, H)),
        ("edge_bih", (G4,)), ("edge_bhh", (G4,)),
        ("seq_Wih", (G4, H)), ("seq_Whh", (G4, H)),
        ("seq_bih", (G4,)), ("seq_bhh", (G4,)),
        ("dec_Wih", (GD, 2 * H)), ("dec_Whh", (GD, EMB)),
        ("dec_bih", (GD,)), ("dec_bhh", (GD,)),
        ("pose_W", (D, SEQ * EMB)), ("pose_b", (D,)),
    ]:
        dummy[n] = (rng.normal(size=s) * 0.1).astype(np.float32)
    out = kernel(**dummy)
    print(out.shape, out.dtype, float(np.abs(out).mean()))


# revision 8
# speedup vs baseline: 1.0381x; 1.0381x over previous
"""Trainium2 Bass kernel for nn_LstmEncDeltaAllHistStacked (v3).

v3 = v2 (1-core, For_i edge loop, persistent compile cache) plus:
  * all inputs packed into ONE DRAM tensor (single device_put per call)
  * node/seq/dec LSTMs also run as hardware loops (8 iters each), with
    h-chain buffers so the s==0 special case disappears (h_prev = 0)
  * pose head as 8 accumulating K=32 matmuls over the dec h-chain
    (replaces the partition-stacked DECP0/DECP1 layout)

Packed input layout (fp32, one row-major [1, NTOT] buffer; offsets in
elements, hardcoded to match _pack_inputs):
  scene_js [2, 2048], scene_sp [3, 2048], w_node_x [3, 256],
  w_node_h [64, 256], w_edge [67, 256], w_seq_x [65, 256],
  w_seq_h [64, 256], w_dec_x [128, 128], w_dec_h [33, 128],
  w_pose_s [32, 16] (col s*2+d = pose_W.T[s-block]), pose_b2 [2, 1]
"""

import os
import numpy as np

NP, SEQ, D, H, EMB = 256, 8, 2, 64, 32
NCORES = 1
PPC = NP
B = PPC * SEQ           # 2048
G4 = 4 * H              # 256
GD = 4 * EMB            # 128
CHUNK = 512
NCH = B // CHUNK        # 4

# packed layout: (name, rows, cols)
_PACK = [
    ("scene_js", D, B),
    ("scene_sp", D + 1, B),
    ("w_node_x", 3, G4),
    ("w_node_h", H, G4),
    ("w_edge", H + 3, G4),
    ("w_seq_x", H + 1, G4),
    ("w_seq_h", H, G4),
    ("w_dec_x", 2 * H, GD),
    ("w_dec_h", EMB + 1, GD),
    ("w_pose_s", EMB, 2 * SEQ),
    ("pose_b2", D, 1),
]
_OFFS = {}
_off = 0
for _n, _r, _c in _PACK:
    _OFFS[_n] = _off
    _off += _r * _c
NTOT = _off

_CACHE = {}


def _enable_jax_compile_cache():
    """Persistent XLA compile cache: run_bass_kernel_spmd rebuilds its jit
    closure per call, so without this every call re-runs the full BIR->NEFF
    compile (~250ms).  Standard jax feature; safe no-op if unavailable."""
    try:
        import jax

        cache_dir = "/tmp/jax_cc_cache"
        os.makedirs(cache_dir, exist_ok=True)
        jax.config.update("jax_compilation_cache_dir", cache_dir)
        jax.config.update("jax_persistent_cache_min_entry_size_bytes", -1)
        jax.config.update("jax_persistent_cache_min_compile_time_secs", 0.0)
    except Exception:
        pass


_enable_jax_compile_cache()


def _build_nc():
    import concourse.bass as bass
    import concourse.tile as tile
    from concourse import bacc, mybir

    f32 = mybir.dt.float32
    AF = mybir.ActivationFunctionType
    OP = mybir.AluOpType

    nc = bacc.Bacc("TRN2", target_bir_lowering=False, debug=False)

    packed_d = nc.dram_tensor("packed_in", [1, NTOT], f32, kind="ExternalInput")
    out_d = nc.dram_tensor("tag_t", [D, PPC], f32, kind="ExternalOutput")

    def pk(name, rows, cols):
        o = _OFFS[name]
        return packed_d[0, o : o + rows * cols].rearrange("(r c) -> r c", c=cols)

    with tile.TileContext(nc) as tc:
        with (
            tc.tile_pool(name="const", bufs=1) as cpool,
            tc.tile_pool(name="state", bufs=1) as spool,
            tc.tile_pool(name="tmp_e", bufs=1) as epool,
            tc.tile_pool(name="tmp_s", bufs=2) as tpool,
        ):
            # ---- load constants (from the packed buffer) ----
            WNX = cpool.tile([3, G4], f32)
            WNH = cpool.tile([H, G4], f32)
            WE = cpool.tile([H + 3, G4], f32)
            WSX = cpool.tile([H + 1, G4], f32)
            WSH = cpool.tile([H, G4], f32)
            WDX = cpool.tile([2 * H, GD], f32)
            WDH = cpool.tile([EMB + 1, GD], f32)
            WPS = cpool.tile([EMB, 2 * SEQ], f32)
            PB = cpool.tile([D, 1], f32)
            SJS = cpool.tile([D, B], f32)
            SLOCE = cpool.tile([3, B], f32)
            for t, (name, rows, cols) in zip(
                [SJS, SLOCE, WNX, WNH, WE, WSX, WSH, WDX, WDH, WPS, PB], _PACK
            ):
                nc.sync.dma_start(t[:], pk(name, rows, cols))

            # ---- persistent state ----
            # CAT rows 0:64 node h (lstm_out), rows 64:128 seq h (full_dist)
            CAT = spool.tile([2 * H, B], f32)
            RHSE = spool.tile([H + 3, B], f32)   # edge rhs: h | x | ones
            EDGEHE = spool.tile([H + 1, B], f32)  # dist_hist | ones
            NODEH = spool.tile([H, (SEQ + 1) * PPC], f32)  # node h chain
            SEQH = spool.tile([H, (SEQ + 1) * PPC], f32)   # seq h chain
            CN = spool.tile([2 * H, PPC], f32)  # c in rows 64:128
            CE = spool.tile([2 * H, B], f32)
            CS = spool.tile([2 * H, PPC], f32)
            CD = spool.tile([4 * EMB, PPC], f32)  # c in rows 32:64
            RHSD = spool.tile([EMB + 1, (SEQ + 1) * PPC], f32)  # dec h | ones
            NEGSLOC = cpool.tile([D, B], f32)

            nc.scalar.mul(NEGSLOC[:], SLOCE[0:2, :], -1.0)
            nc.gpsimd.memset(RHSE[0:H, :], 0.0)
            nc.sync.dma_start(RHSE[H + 2 : H + 3, :], SLOCE[2:3, :])
            nc.gpsimd.memset(EDGEHE[H : H + 1, :], 1.0)
            nc.gpsimd.memset(NODEH[:, 0:PPC], 0.0)
            nc.gpsimd.memset(SEQH[:, 0:PPC], 0.0)
            nc.gpsimd.memset(CN[H : 2 * H, :], 0.0)
            nc.gpsimd.memset(CE[H : 2 * H, :], 0.0)
            nc.gpsimd.memset(CS[H : 2 * H, :], 0.0)
            nc.gpsimd.memset(CD[EMB : 2 * EMB, :], 0.0)
            nc.gpsimd.memset(RHSD[:, 0:PPC], 0.0)
            nc.gpsimd.memset(RHSD[EMB : EMB + 1, :], 1.0)

            def small_lstm_loop(WX, WH, HCH, Cst, xs_of, tag):
                """8-step LSTM as a hardware loop; h chain in HCH
                ([H, 9*PPC], slice 0 zeroed), gates via the all-sigmoid
                trick.  xs_of(iv) -> x-slice [Kx, PPC] for step iv//PPC."""
                S = tpool.tile([2 * H, 2 * PPC], f32, tag=tag + "s")
                Q = tpool.tile([2 * H, PPC], f32, tag=tag + "q")
                P1 = tpool.tile([2 * H, PPC], f32, tag=tag + "p1")
                P2 = tpool.tile([2 * H, PPC], f32, tag=tag + "p2")
                TH = tpool.tile([2 * H, PPC], f32, tag=tag + "th")
                GP = tpool.tile_psum([2 * H, 2 * PPC], f32, tag=tag + "g")
                c = Cst[H : 2 * H, :]
                with tc.For_i(0, SEQ * PPC, PPC) as iv:
                    rx = xs_of(iv)
                    rh = HCH[:, bass.ds(iv, PPC)]
                    for mh in range(2):
                        o = GP[:, mh * PPC : (mh + 1) * PPC]
                        nc.tensor.matmul(
                            o, WX[:, mh * 128 : (mh + 1) * 128], rx,
                            start=True, stop=False,
                        )
                        nc.tensor.matmul(
                            o, WH[:, mh * 128 : (mh + 1) * 128], rh,
                            start=False, stop=True,
                        )
                    nc.scalar.activation(S[:], GP[:], AF.Sigmoid)
                    si, sf = S[0:H, 0:PPC], S[H : 2 * H, 0:PPC]
                    sg = S[0:H, PPC : 2 * PPC]
                    so = S[H : 2 * H, PPC : 2 * PPC]
                    nc.vector.tensor_mul(Q[0:H, :], si, sg)
                    nc.vector.scalar_tensor_tensor(
                        P1[0:H, :], Q[0:H, :], 2.0, si,
                        op0=OP.mult, op1=OP.subtract,
                    )
                    nc.vector.tensor_mul(P2[0:H, :], sf, c)
                    nc.vector.tensor_add(c, P1[0:H, :], P2[0:H, :])
                    nc.scalar.activation(TH[H : 2 * H, :], c, AF.Tanh)
                    nc.vector.tensor_mul(
                        HCH[:, bass.ds(iv + PPC, PPC)], so, TH[H : 2 * H, :]
                    )

            # ======== node LSTM (batch 256, hw loop over 8 steps) ========
            with tc.tile_pool(
                name="ps_n", bufs=1, space=bass.MemorySpace.PSUM
            ) as ps_n:
                tpool.tile_psum = (
                    lambda shape, dt, tag: ps_n.tile(shape, dt, tag=tag, name=tag)
                )
                small_lstm_loop(
                    WNX, WNH, NODEH, CN,
                    lambda iv: SLOCE[:, bass.ds(iv, PPC)], "n",
                )
                # lstm_out -> CAT rows 0:64
                nc.vector.tensor_copy(CAT[0:H, :], NODEH[:, PPC:])

            # ======== edge LSTM (batch 2048, hw loop over 256 steps) =====
            with tc.tile_pool(
                name="ps_e", bufs=1, space=bass.MemorySpace.PSUM
            ) as ps_e:
                GE = ps_e.tile([2 * H, 2 * B], f32)
                S = epool.tile([2 * H, 2 * B], f32)
                Q = epool.tile([2 * H, B], f32)
                P1 = epool.tile([2 * H, B], f32)
                P2 = epool.tile([2 * H, B], f32)
                TH = epool.tile([2 * H, B], f32)
                c = CE[H : 2 * H, :]

                def edge_body(iv):
                    nc.vector.tensor_add(
                        RHSE[H : H + 2, :].rearrange("d (s p) -> d s p", p=PPC),
                        SJS[:, bass.ds(iv, SEQ)]
                        .unsqueeze(2)
                        .broadcast_to((D, SEQ, PPC)),
                        NEGSLOC[:].rearrange("d (s p) -> d s p", p=PPC),
                    )
                    for ch in range(NCH):
                        rc = RHSE[:, ch * CHUNK : (ch + 1) * CHUNK]
                        for mh in range(2):
                            nc.tensor.matmul(
                                GE[:, mh * B + ch * CHUNK : mh * B + (ch + 1) * CHUNK],
                                WE[:, mh * 128 : (mh + 1) * 128],
                                rc,
                                start=True, stop=True,
                            )
                    nc.scalar.activation(S[:], GE[:], AF.Sigmoid)
                    si, sf = S[0:H, 0:B], S[H : 2 * H, 0:B]
                    sg, so = S[0:H, B : 2 * B], S[H : 2 * H, B : 2 * B]
                    nc.vector.tensor_mul(Q[0:H, :], si, sg)
                    nc.vector.scalar_tensor_tensor(
                        P1[0:H, :], Q[0:H, :], 2.0, si,
                        op0=OP.mult, op1=OP.subtract,
                    )
                    nc.vector.tensor_mul(P2[0:H, :], sf, c)
                    nc.vector.tensor_add(c, P1[0:H, :], P2[0:H, :])
                    nc.scalar.activation(TH[H : 2 * H, :], c, AF.Tanh)
                    nc.vector.tensor_mul(RHSE[0:H, :], so, TH[H : 2 * H, :])

                tc.For_i_unrolled(0, B, SEQ, edge_body, max_unroll=4)
                nc.vector.tensor_copy(EDGEHE[0:H, :], RHSE[0:H, :])

            # ======== seq LSTM (batch 256, hw loop over 8 steps) =========
            with tc.tile_pool(
                name="ps_s", bufs=1, space=bass.MemorySpace.PSUM
            ) as ps_s:
                tpool.tile_psum = (
                    lambda shape, dt, tag: ps_s.tile(shape, dt, tag=tag, name=tag)
                )
                small_lstm_loop(
                    WSX, WSH, SEQH, CS,
                    lambda iv: EDGEHE[:, bass.ds(iv, PPC)], "e",
                )
                # full_dist -> CAT rows 64:128 (partition remap via DMA)
                nc.sync.dma_start(CAT[H : 2 * H, :], SEQH[:, PPC:])

                # ======== decoder LSTM (hw loop, H=EMB=32) ========
                SD = tpool.tile([4 * EMB, PPC], f32, tag="dsif")
                TGSO = tpool.tile([4 * EMB, PPC], f32, tag="dtgso")
                DP1 = tpool.tile([4 * EMB, PPC], f32, tag="dp1")
                DP2 = tpool.tile([4 * EMB, PPC], f32, tag="dp2")
                DTH = tpool.tile([4 * EMB, PPC], f32, tag="dth")
                GDm = ps_s.tile([GD, PPC], f32, tag="gdec")
                cd = CD[EMB : 2 * EMB, :]
                with tc.For_i(0, SEQ * PPC, PPC) as iv:
                    nc.tensor.matmul(
                        GDm[:], WDX[:], CAT[:, bass.ds(iv, PPC)],
                        start=True, stop=False,
                    )
                    nc.tensor.matmul(
                        GDm[:], WDH[:], RHSD[:, bass.ds(iv, PPC)],
                        start=False, stop=True,
                    )
                    nc.scalar.activation(
                        SD[0 : 2 * EMB, :], GDm[0 : 2 * EMB, :], AF.Sigmoid
                    )
                    nc.scalar.activation(
                        TGSO[0:EMB, :], GDm[2 * EMB : 3 * EMB, :], AF.Tanh
                    )
                    nc.scalar.activation(
                        TGSO[EMB : 2 * EMB, :], GDm[3 * EMB : 4 * EMB, :],
                        AF.Sigmoid,
                    )
                    nc.vector.tensor_mul(
                        DP1[0:EMB, :], SD[0:EMB, :], TGSO[0:EMB, :]
                    )
                    nc.vector.tensor_mul(
                        DP2[0:EMB, :], SD[EMB : 2 * EMB, :], cd
                    )
                    nc.vector.tensor_add(cd, DP1[0:EMB, :], DP2[0:EMB, :])
                    nc.scalar.activation(DTH[EMB : 2 * EMB, :], cd, AF.Tanh)
                    nc.vector.tensor_mul(
                        RHSD[0:EMB, bass.ds(iv + PPC, PPC)],
                        TGSO[EMB : 2 * EMB, :],
                        DTH[EMB : 2 * EMB, :],
                    )

                # ======== pose head: 8 accumulating K=32 matmuls ========
                TAGT = ps_s.tile([D, PPC], f32, tag="tag")
                for s in range(SEQ):
                    nc.tensor.matmul(
                        TAGT[:],
                        WPS[:, 2 * s : 2 * (s + 1)],
                        RHSD[0:EMB, (s + 1) * PPC : (s + 2) * PPC],
                        start=(s == 0), stop=(s == SEQ - 1),
                    )
                OUTT = tpool.tile([D, PPC], f32, tag="outt")
                nc.vector.scalar_tensor_tensor(
                    OUTT[:], TAGT[:], PB[:],
                    SLOCE[0:2, (SEQ - 1) * PPC : SEQ * PPC],
                    op0=OP.add, op1=OP.add,
                )
                nc.sync.dma_start(out_d[:], OUTT[:])

    nc.compile()
    return nc


def _prep_weights(i):
    """Host-side constant folding of the LSTM weights into matmul layouts."""
    c = np.concatenate
    f = np.float32
    wnx = c([i["node_Wih"].T, (i["node_bih"] + i["node_bhh"])[None]], 0).copy()
    wnh = i["node_Whh"].T.copy()
    wnx[:, 128:192] *= 2.0
    wnh[:, 128:192] *= 2.0
    we = c([i["edge_Whh"].T, i["edge_Wih"].T,
            (i["edge_bih"] + i["edge_bhh"])[None]], 0)
    we = we.copy()
    we[:, 128:192] *= 2.0  # g-gate cols: tanh(g) = 2*sigmoid(2g) - 1
    wsx = c([i["seq_Wih"].T, (i["seq_bih"] + i["seq_bhh"])[None]], 0).copy()
    wsh = i["seq_Whh"].T.copy()
    wsx[:, 128:192] *= 2.0
    wsh[:, 128:192] *= 2.0
    wdx = i["dec_Wih"].T
    wdh = c([i["dec_Whh"].T, (i["dec_bih"] + i["dec_bhh"])[None]], 0)
    # pose_W [2, 256] -> per-step blocks: w_pose_s[e, s*2+d] = pose_W[d, s*32+e]
    wps = np.ascontiguousarray(
        i["pose_W"].reshape(2, SEQ, EMB).transpose(2, 1, 0).reshape(EMB, 2 * SEQ)
    )
    pb = i["pose_b"][:, None]
    return {
        "w_node_x": np.ascontiguousarray(wnx, f),
        "w_node_h": np.ascontiguousarray(wnh, f),
        "w_edge": np.ascontiguousarray(we, f),
        "w_seq_x": np.ascontiguousarray(wsx, f),
        "w_seq_h": np.ascontiguousarray(wsh, f),
        "w_dec_x": np.ascontiguousarray(wdx, f),
        "w_dec_h": np.ascontiguousarray(wdh, f),
        "w_pose_s": wps.astype(f),
        "pose_b2": np.ascontiguousarray(pb, f),
    }


def make_in_maps(**inputs):
    scene = np.ascontiguousarray(np.asarray(inputs["scene"], np.float32))
    w = _prep_weights({k: np.asarray(v, np.float32) for k, v in inputs.items()})
    w["scene_js"] = np.ascontiguousarray(scene.transpose(2, 0, 1).reshape(D, B))
    ssp = scene.transpose(2, 1, 0).reshape(D, B)
    w["scene_sp"] = np.ascontiguousarray(
        np.concatenate([ssp, np.ones((1, B), np.float32)], 0)
    )
    packed = np.empty((1, NTOT), np.float32)
    for name, rows, cols in _PACK:
        o = _OFFS[name]
        packed[0, o : o + rows * cols] = w[name].reshape(-1)
    return [{"packed_in": packed}]


def gather_out(results):
    out = np.zeros((NP, 1, D), np.float32)
    out[:, 0, :] = results[0]["tag_t"].T
    return out


def kernel(**inputs):
    from concourse.bass_utils import run_bass_kernel_spmd

    first = "nc" not in _CACHE
    if first:
        _CACHE["nc"] = _build_nc()
    nc = _CACHE["nc"]
    in_maps = make_in_maps(**inputs)
    res = run_bass_kernel_spmd(nc, in_maps, list(range(NCORES)))
    if first:
        # Warm the dispatch path (compile cache, executable-load dedup in
        # the PJRT client/terminal): per-call latency settles only after a
        # few calls in a fresh process.
        for _ in range(4):
            run_bass_kernel_spmd(nc, in_maps, list(range(NCORES)))
    return gather_out(res.results)


if __name__ == "__main__":
    rng = np.random.default_rng(0)
    dummy = {}
    dummy["scene"] = rng.normal(size=(NP, SEQ, D)).astype(np.float32)
    for n, s in [
        ("node_Wih", (G4, D)), ("node_Whh", (G4, H)),
        ("node_bih", (G4,)), ("node_bhh", (G4,)),
        ("edge_Wih", (G4, D)), ("edge_Whh", (G4, H)),
        ("edge_bih", (G4,)), ("edge_bhh", (G4,)),
        ("seq_Wih", (G4, H)), ("seq_Whh", (G4, H)),
        ("seq_bih", (G4,)), ("seq_bhh", (G4,)),
        ("dec_Wih", (GD, 2 * H)), ("dec_Whh", (GD, EMB)),
        ("dec_bih", (GD,)), ("dec_bhh", (GD,)),
        ("pose_W", (D, SEQ * EMB)), ("pose_b", (D,)),
    ]:
        dummy[n] = (rng.normal(size=s) * 0.1).astype(np.float32)
    out = kernel(**dummy)
    print(out.shape, out.dtype, float(np.abs(out).mean()))
